# revision 46
# baseline (speedup 1.0000x reference)
"""Trainium2 Bass kernel for MultiHeadSelfAttention (B=8, C=512, H=W=32, 8 heads).

Sharding: data-parallel - one batch element per NeuronCore (8 cores).

v3: Act-engine-bound design. The 64 exp activations ([128,1024] each) are the
hard floor (~64us busy); everything else is cut or overlapped under them:
  - Q/K/V/out projections in fp8-e4m3 DoubleRow on a pre-centered (x - mu)
    fp8 copy (no rank-1 mu-fold matmuls; mu broadcast via one PE matmul,
    centering on the Pool engine).
  - PV in fp8 DoubleRow: exp writes fp8 probs into [128, 2, S] token-chunk
    pair tiles; v stored fp8 as [p, pair, slot, head, 66] with sigma-aug
    columns 64/65. pt carries 2^-k (k = -exponent(rsqrt(var)), a power of two
    exact in fp8) so the aug column (2^k) cancels exactly in the softmax
    denominator; v8 carries the residual r*2^k scale.
  - LayerNorm r = rsqrt(var) computed WITHOUT the Act engine: variance rows
    are PE-transposed to [128,4] columns, then a quake-style bit seed + two
    Newton steps run as tiny DVE ops; 2^k/k come from the exponent bits.
    The Act engine therefore runs Exp only - zero activation-table reloads
    and the exp stream is never interrupted.
  - All stats transposes are tiny PE is_transpose matmuls; no DRAM scratch.
  - Head pair-slot layout [64 d, pair, slot] keeps PV output, sigma
    normalization and the output projection partition-aligned: no
    SBUF->SBUF partition-move DMAs. Sigma rides PV rows 64/65, reciprocated
    on DVE and broadcast by one tiny PE matmul per head.
  - The last pair group's PV + drain are deferred into the next group's
    first score slot so the exp stream never waits on PV/normalization.
  - gpsimd DMAs are SWDGE (burn Pool-engine time): only the y stores ride
    the gpsimd queue; x/weight loads use the SP/Act hardware DGE queues,
    ordered x-half0, x-half1, wqk, wv, wo for the serial HBM stream.
  - In chained (repeat) mode each y chunk is copied into the next body's
    input tile as soon as it exists, so the next layer's stats overlap this
    layer's second-half attention (the body boundary costs ~0 Act idle).
"""

import math

import numpy as np

C = 512
S = 1024
B = 8
NH = 8
HD = 64
N_CORES = 8

LN2 = float(math.log(2.0))
MAGIC = float(2.0 ** 23)
QMAGIC = 0x5F3759DF

_CACHE = {}


def _build_nc(repeat=1, debug=False):
    import concourse.bass as bass
    import concourse.mybir as mybir
    import concourse.tile as tile
    from concourse import bacc, masks

    f32 = mybir.dt.float32
    f32r = mybir.dt.float32r
    bf16 = mybir.dt.bfloat16
    fp8 = mybir.dt.float8e4
    i32 = mybir.dt.int32
    AF = mybir.ActivationFunctionType
    OP = mybir.AluOpType
    PM = mybir.MatmulPerfMode

    nc = bacc.Bacc("TRN2", debug=False, num_devices=N_CORES)

    x_d = nc.declare_dram_parameter("x", [C, S], f32r, isOutput=False)
    wqk_d = nc.declare_dram_parameter("wqk", [128, 2, 2, 2 * C], fp8, isOutput=False)
    wv_d = nc.declare_dram_parameter("wv", [128, 2, 2, C], fp8, isOutput=False)
    wo_d = nc.declare_dram_parameter("wo", [64, 4, 2, C], fp8, isOutput=False)
    csts_d = nc.declare_dram_parameter("csts", [128, 9], f32r, isOutput=False)
    y_d = nc.declare_dram_parameter("y", [C, S], f32, isOutput=True)

    def r32(ap):
        return ap.bitcast(f32r)

    with tile.TileContext(nc) as tc:
        import contextlib

        with contextlib.ExitStack() as ctx:
            ctx.enter_context(nc.allow_low_precision(reason="fp8/bf16 attention"))
            const = ctx.enter_context(tc.tile_pool(name="const", bufs=1))
            big = ctx.enter_context(tc.tile_pool(name="big", bufs=1))
            xsq_pool = ctx.enter_context(tc.tile_pool(name="xsq", bufs=6))
            pt_pool = ctx.enter_context(tc.tile_pool(name="pt", bufs=6))
            ost_pool = ctx.enter_context(tc.tile_pool(name="ost", bufs=6))
            y_pool = ctx.enter_context(tc.tile_pool(name="ysb", bufs=4 if repeat == 1 else 8))
            stats_sb = ctx.enter_context(tc.tile_pool(name="stats_sb", bufs=1))
            sg_pool = ctx.enter_context(tc.tile_pool(name="sg", bufs=2))
            ps = ctx.enter_context(tc.tile_pool(name="ps", bufs=2, space="PSUM"))

            # ---- static loads (x first, chunked, so stats start early) --------
            xpool = ctx.enter_context(tc.tile_pool(name="xpool", bufs=1 if repeat == 1 else 2))
            xsb = xpool.tile([128, 4, S], f32r, tag="xping")
            x_re = x_d[:, :].rearrange("(kc p) s -> p kc s", p=128)
            wqk_sb = const.tile([128, 2, 2, 2 * C], fp8)
            wv_sb = const.tile([128, 2, 2, C], fp8)
            wo_sb = const.tile([64, 4, 2, C], fp8)
            csts_sb = const.tile([128, 9], f32r)

            def xdma(eng, kc, sc):
                eng.dma_start(
                    out=xsb[:, kc, sc * 512:(sc + 1) * 512],
                    in_=x_re[:, kc, sc * 512:(sc + 1) * 512])

            # Issue order tuned for the serial HBM stream: x half 0 first,
            # then k/q weights, then x half 1, then wv/wo. gpsimd DMAs are
            # SWDGE (cost Pool-engine time) so only 2 x chunks ride there.
            nc.scalar.dma_start(out=csts_sb[:], in_=csts_d[:, :])
            xdma(nc.sync, 0, 0)
            xdma(nc.scalar, 1, 0)
            xdma(nc.scalar, 2, 0)
            xdma(nc.sync, 3, 0)
            xdma(nc.scalar, 1, 1)
            xdma(nc.scalar, 2, 1)
            xdma(nc.sync, 0, 1)
            xdma(nc.sync, 3, 1)
            nc.scalar.dma_start(out=wqk_sb[:, :, :, 512:], in_=wqk_d[:, :, :, 512:])
            nc.sync.dma_start(out=wqk_sb[:, :, :, 0:512], in_=wqk_d[:, :, :, 0:512])
            nc.scalar.dma_start(out=wv_sb[:], in_=wv_d[:, :, :, :])
            nc.sync.dma_start(out=wo_sb[:], in_=wo_d[:, :, :, :])

            # PE p-state warm-up fuel: memset early (on DVE; Pool may be
            # busy with SWDGE x transfers) so the warm chain starts at t~1us
            warmmov = const.tile([128, 512], bf16)
            nc.vector.memset(warmmov[:], 0.0)
            onesb = const.tile([128, 1], bf16)
            nc.vector.memset(onesb[:], 1.0)
            ones1f = const.tile([1, 128], f32)
            nc.vector.memset(ones1f[:], 1.0)
            ones1 = const.tile([1, 128], f32r)
            nc.vector.tensor_copy(ones1[:], ones1f[:])
            onescf = const.tile([128, 1], f32)
            nc.vector.memset(onescf[:], 1.0)
            onesc = const.tile([128, 1], f32r)
            nc.vector.tensor_copy(onesc[:], onescf[:])
            # row of ones at partition 64: stationary for the sigma broadcast
            e64f = const.tile([65, 64], f32)
            nc.gpsimd.memset(e64f[64:65, :], 1.0)
            e64 = const.tile([65, 64], f32r)
            nc.vector.tensor_copy(e64[64:65, :], e64f[64:65, :])
            # 1x1 identity for PE row->column transposes
            ident1 = const.tile([1, 1], f32)
            nc.gpsimd.memset(ident1[:], 1.0)
            # 128x128 identity for column->row transposes
            idN = const.tile([128, 128], f32)
            masks.make_identity(nc, idN[:])
            # aug multiplier: writes the sigma columns 64,65 of v8
            augf2 = const.tile([128, 8, 2], f32)
            nc.vector.memset(augf2[:], 1.0)
            epsr = const.tile([1, 1], f32)
            nc.vector.memset(epsr[:], 1e-5)
            # prime the ln/exp table during idle startup
            warm = const.tile([1, 1], f32)
            nc.scalar.activation(warm[:], ones1f[0:1, 0:1], AF.Exp,
                                 bias=0.0, scale=1.0)
            # PE p-state warm-up: keep the tensor engine continuously busy
            # during the x load so the stats chain runs at full clock

            def psA(name):
                return ps.tile([128, S], f32, tag="psA", name=name)

            def psB(name):
                return ps.tile([128, 512], f32, tag="psB", name=name)

            def psBt(shape, name):
                return ps.tile(shape, f32, tag="psB", name=name)

            def psC(name):
                return ps.tile([128, 512], f32, tag="psC", name=name)

            warm_ps = ps.tile([128, S], f32, tag="psA", name="warm_ps")
            for w in range(4):
                nc.tensor.matmul(warm_ps[0:1, 0:512], onesb[:], warmmov[:],
                                 start=(w == 0), stop=(w == 3))

            def body(src, dst_dram, it, nxt=None):
                """One attention layer: src(kc, sc) -> writes dst_dram.
                When chaining (nxt given), each y chunk is copied into nxt
                as soon as it exists so the next body's stats can start
                while this body's second-half attention still runs."""
                x8 = big.tile([128, 2, 2, S], fp8, tag="x8")
                murow = stats_sb.tile([1, S], f32r, tag="murow")
                srowA = stats_sb.tile([1, S], f32, tag="srowA")
                srowB = stats_sb.tile([1, S], f32, tag="srowB")
                rrowS = stats_sb.tile([1, S], f32r, tag="rrowS")
                g1row = stats_sb.tile([1, S], f32, tag="g1row")
                kf = stats_sb.tile([128, 8], f32, tag="kf")
                sqcol = stats_sb.tile([128, 8], f32, tag="sqcol")
                rcol2 = stats_sb.tile([128, 8], f32, tag="rcol2")
                scol = stats_sb.tile([128, 8], f32, tag="scol")
                gt = stats_sb.tile([128, 8], f32, tag="gt")
                gcol = stats_sb.tile([128, 8], f32, tag="gcol")
                varE = stats_sb.tile([128, 8], f32, tag="varE")
                nw1 = stats_sb.tile([128, 8], f32, tag="nw1")
                nw2 = stats_sb.tile([128, 8], f32, tag="nw2")
                nw3 = stats_sb.tile([128, 8], f32, tag="nw3")
                R_sb = big.tile([128, S], f32, tag="R")
                M_sb = big.tile([128, S], f32, tag="M")

                # ---- per-token-half stats, staged so half 1 can be emitted
                # interleaved into the early attention stream (its x lands
                # late; emitting it upfront would block the in-order PE/DVE
                # queues behind the DMA wait) -------------------------------
                stq_rows = {}

                def stats_stage1(sc):
                    """sums -> mu -> MU broadcast -> x8 centering."""
                    h = slice(sc * 512, (sc + 1) * 512)
                    sts_t = psB(f"stx{it}_{sc}")
                    stq_t = psB(f"stq{it}_{sc}")
                    sts = sts_t[0:1, :]
                    for kc in range(4):
                        nc.tensor.matmul(
                            sts, r32(onesc[:]), r32(src(kc, sc)),
                            start=(kc == 0), stop=(kc == 3))
                    xsqs = []
                    for kc in range(4):
                        xs = src(kc, sc)
                        xsq = xsq_pool.tile([128, 512], f32r)
                        if kc < 2:
                            nc.gpsimd.tensor_mul(xsq[:], xs.bitcast(f32),
                                                 xs.bitcast(f32))
                        xsqs.append(xsq)
                    nc.vector.tensor_scalar_mul(murow[:, h], sts, 1.0 / C)
                    for kc in range(2, 4):
                        xs = src(kc, sc)
                        nc.gpsimd.tensor_mul(xsqs[kc][:], xs.bitcast(f32),
                                             xs.bitcast(f32))
                    ps_m = psB(f"ps_m{it}_{sc}")
                    nc.tensor.matmul(ps_m[:], r32(ones1[:]), r32(murow[:, h]),
                                     start=True, stop=True)
                    nc.vector.tensor_copy(M_sb[:, h], ps_m[:])
                    stq = stq_t[0:1, :]
                    for kc in range(4):
                        nc.tensor.matmul(
                            stq, r32(onesc[:]), r32(xsqs[kc][:]),
                            start=(kc == 0), stop=(kc == 3))
                    stq_rows[sc] = stq
                    for kc in range(4):
                        nc.gpsimd.tensor_tensor(
                            x8[:, kc // 2, kc % 2, h], src(kc, sc).bitcast(f32),
                            M_sb[:, h], OP.subtract)

                def stats_stage2(sc):
                    """variance -> transposed rsqrt (quake seed + 2 Newton
                    steps, all DVE bit ops; the Act engine stays Exp-only,
                    so the big-exp stream is never interrupted by table
                    loads) -> r row via back-transpose -> R broadcast."""
                    h = slice(sc * 512, (sc + 1) * 512)
                    cs = slice(4 * sc, 4 * sc + 4)
                    stq = stq_rows[sc]
                    nc.vector.tensor_mul(srowB[:, h], murow[:, h].bitcast(f32),
                                         murow[:, h].bitcast(f32))
                    nc.vector.scalar_tensor_tensor(
                        srowA[:, h], stq, 1.0 / C, srowB[:, h],
                        OP.mult, OP.subtract)
                    psT = psBt([128, 4, 2], f"psT{it}_{sc}")
                    for c in range(4):
                        c0 = sc * 512 + c * 128
                        nc.tensor.transpose(
                            psT[:, c, 0:1], srowA[0:1, c0:c0 + 128], ident1[:])
                    nc.vector.tensor_scalar_add(varE[:, cs], psT[:, :, 0], 1e-5)
                    vi = varE[:, cs].bitcast(i32)
                    nc.vector.tensor_scalar(nw1[:, cs].bitcast(i32), vi,
                                            1, None, OP.arith_shift_right)
                    # y0 bits = QMAGIC - (v>>1)
                    nc.vector.tensor_scalar(nw2[:, cs].bitcast(i32),
                                            nw1[:, cs].bitcast(i32),
                                            -1, QMAGIC, OP.mult, OP.add)
                    y0 = nw2[:, cs]
                    nc.vector.tensor_mul(nw3[:, cs], y0, y0)
                    nc.vector.tensor_mul(nw1[:, cs], nw3[:, cs], varE[:, cs])
                    nc.vector.tensor_scalar(nw3[:, cs], nw1[:, cs], -0.5, 1.5,
                                            OP.mult, OP.add)
                    nc.vector.tensor_mul(nw1[:, cs], y0, nw3[:, cs])
                    nc.vector.tensor_mul(nw2[:, cs], nw1[:, cs], nw1[:, cs])
                    nc.vector.tensor_mul(nw3[:, cs], nw2[:, cs], varE[:, cs])
                    nc.vector.tensor_scalar(nw2[:, cs], nw3[:, cs], -0.5, 1.5,
                                            OP.mult, OP.add)
                    nc.vector.tensor_mul(rcol2[:, cs], nw1[:, cs], nw2[:, cs])
                    psRow = psBt([1, 512], f"psRw{it}_{sc}")
                    for c in range(4):
                        nc.tensor.transpose(
                            psRow[0:1, c * 128:(c + 1) * 128],
                            rcol2[:, 4 * sc + c:4 * sc + c + 1], idN[:])
                    nc.vector.tensor_copy(rrowS[:, h], psRow[:])
                    ps_r = psB(f"ps_r{it}_{sc}")
                    nc.tensor.matmul(ps_r[:], r32(ones1[:]), rrowS[:, h],
                                     start=True, stop=True)
                    nc.vector.tensor_copy(R_sb[:, h], ps_r[:])

                def stats_stage3(sc):
                    """g1 row + transposed exp-bias cols; 2^k from the rsqrt
                    exponent bits (exact in fp8, cancels in sigma)."""
                    h = slice(sc * 512, (sc + 1) * 512)
                    cs = slice(4 * sc, 4 * sc + 4)
                    gps_t = psB(f"g{it}_{sc}")
                    gps = gps_t[0:1, :]
                    for kc in range(4):
                        nc.tensor.matmul(
                            gps, csts_sb[:, kc:kc + 1], r32(src(kc, sc)),
                            start=(kc == 0), stop=(kc == 3))
                    nc.vector.scalar_tensor_tensor(
                        g1row[:, h], murow[:, h].bitcast(f32), csts_sb[0:1, 8:9].bitcast(f32), gps,
                        OP.mult, OP.add)
                    psT = psBt([128, 4, 2], f"psTg{it}_{sc}")
                    for c in range(4):
                        c0 = sc * 512 + c * 128
                        nc.tensor.transpose(
                            psT[:, c, 1:2], g1row[0:1, c0:c0 + 128], ident1[:])
                    # k = -exponent(rcol2): 2^k and k exact from the bits
                    ri = rcol2[:, cs].bitcast(i32)
                    nc.vector.tensor_scalar(nw1[:, cs].bitcast(i32), ri,
                                            23, None, OP.arith_shift_right)
                    nc.vector.tensor_scalar(nw2[:, cs].bitcast(i32),
                                            nw1[:, cs].bitcast(i32),
                                            -1, 127, OP.mult, OP.add)
                    nc.vector.tensor_copy(kf[:, cs], nw2[:, cs].bitcast(i32))
                    nc.vector.tensor_scalar(nw3[:, cs].bitcast(i32),
                                            nw1[:, cs].bitcast(i32),
                                            -1, 254, OP.mult, OP.add)
                    nc.vector.tensor_scalar(sqcol[:, cs].bitcast(i32),
                                            nw3[:, cs].bitcast(i32),
                                            23, None, OP.arith_shift_left)
                    # v scale = r * 2^k (exact-cancel form of aug)
                    nc.vector.tensor_mul(scol[:, cs], sqcol[:, cs], rcol2[:, cs])
                    # exp bias: gcol = -k*ln2 + 0.125*r*g1
                    nc.vector.scalar_tensor_tensor(
                        gt[:, cs], psT[:, :, 1], 0.125, rcol2[:, cs],
                        OP.mult, OP.mult)
                    nc.vector.scalar_tensor_tensor(
                        gcol[:, cs], kf[:, cs], -LN2, gt[:, cs],
                        OP.mult, OP.add)

                stats_stage1(0)
                stats_stage2(0)
                stats_stage3(0)
                stats_stage1(1)
                stats_stage2(1)
                stats_stage3(1)

                # ---- QK projections (fp8 DoubleRow on centered x8) ------------
                qt_sb = big.tile([128, 4, S], bf16, tag="qt")
                kt_sb = big.tile([128, 4, S], bf16, tag="kt")

                def qk_half(oc, sc, pool=None):
                    dst = qt_sb if oc < 4 else kt_sb
                    o4 = oc % 4
                    h = slice(sc * 512, (sc + 1) * 512)
                    p = (pool or psB)(f"qk{it}_{oc}_{sc}")
                    for g in range(2):
                        nc.tensor.matmul(
                            p[:],
                            wqk_sb[:, g, :, oc * 128:(oc + 1) * 128],
                            x8[:, g, :, h],
                            start=(g == 0), stop=(g == 1),
                            perf_mode=PM.DoubleRow,
                        )
                    nc.vector.tensor_tensor(
                        dst[:, o4, h], p[:], R_sb[:, h], OP.mult)

                qk_half(4, 0, pool=psC)
                qk_half(0, 0)

                # v8[p_tok, pair, slot, head, 0:64] = fp8(v_raw * r * 2^k)
                # cols 64,65 = 2^k (sigma aug, exact in fp8; 66-wide keeps
                # the stationary AP run even for the ldweights ISA check)
                v8 = big.tile([128, 4, 2, 8, 66], fp8, tag="v8")

                def v_chunk(tcn):
                    j, sl = tcn // 2, tcn % 2
                    p = psB(f"v{it}_{tcn}")
                    for g in range(2):
                        nc.tensor.matmul(
                            p[:],
                            x8[:, g, :, tcn * 128:(tcn + 1) * 128],
                            wv_sb[:, g, :, :],
                            start=(g == 0), stop=(g == 1),
                            perf_mode=PM.DoubleRow,
                        )
                    nc.vector.tensor_scalar_mul(
                        v8[:, j, sl, :, 0:64],
                        p[:].rearrange("p (h d) -> p h d", h=8),
                        scol[:, tcn:tcn + 1])
                    nc.gpsimd.tensor_scalar_mul(
                        v8[:, j, sl, :, 64:66], augf2[:], sqcol[:, tcn:tcn + 1])

                # ---- attention: sc-major; per (sc, pr): 4 pair groups ---------
                # opk8[d, pair(pr), slot(hi), s]: normalized attn out, fp8
                opk8 = big.tile([64, 4, 2, S], fp8, tag="opk8")
                pending = []  # (pr, sc, sg, osts)

                def emit_norm(tail=False):
                    pr, sc, sg, osts = pending.pop(0)
                    h = slice(sc * 512, (sc + 1) * 512)
                    for hi in range(2):
                        ps_e = psB(f"pe{it}_{pr}_{sc}_{hi}")
                        nc.tensor.matmul(
                            ps_e[0:64, :],
                            e64[64:65, :],
                            sg[64:65, hi * 512:(hi + 1) * 512],
                            start=True, stop=True,
                        )
                        nc.vector.tensor_tensor(
                            opk8[:, pr, hi, h], osts[hi][:],
                            ps_e[0:64, :], OP.mult)

                def y_chunk(c, sc):
                    h = slice(sc * 512, (sc + 1) * 512)
                    ps_y = psB(f"y{it}_{c}_{sc}")
                    for j in range(4):
                        nc.tensor.matmul(
                            ps_y[:],
                            wo_sb[:, j, :, c * 128:(c + 1) * 128],
                            opk8[:, j, :, h],
                            start=(j == 0), stop=(j == 3),
                            perf_mode=PM.DoubleRow,
                        )
                    ysb = y_pool.tile([128, 512], f32)
                    nc.vector.scalar_tensor_tensor(
                        ysb[:], ps_y[:], csts_sb[:, 4 + c:5 + c].bitcast(f32),
                        src(c, sc).bitcast(f32),
                        OP.add, OP.add,
                    )
                    qeng = [nc.sync, nc.gpsimd, nc.sync, nc.gpsimd][c]
                    qeng.dma_start(
                        out=dst_dram[c * 128:(c + 1) * 128, h],
                        in_=ysb[:],
                    )
                    if nxt is not None:
                        nc.gpsimd.tensor_copy(nxt[:, c, h], ysb[:])
                    out_tiles.append(ysb)

                # deferred last-pair PV: emitted just after the next group's
                # first score matmuls so the Act exp stream never waits
                deferred = []

                def flush_deferred():
                    while deferred:
                        deferred.pop(0)()

                out_tiles = []
                for sc in range(2):
                    for pr in range(4):
                        sg = sg_pool.tile([65, S], f32r, tag="sg",
                                          name=f"sg{it}_{pr}_{sc}")
                        pvp = [psC(f"pv{it}_{pr}_{sc}_{hi}") for hi in range(2)]
                        for j in range(4):
                            pt2 = pt_pool.tile([128, 2, S], fp8)
                            for sl in range(2):
                                tcn = 2 * j + sl
                                if pr == 0 and sc == 0:
                                    v_chunk(tcn)
                                pst = psA(f"sc{it}_{pr}_{sc}_{tcn}")
                                for hi in range(2):
                                    b0 = 64 * hi
                                    nc.tensor.matmul(
                                        pst[:, hi * 512:(hi + 1) * 512],
                                        kt_sb[b0:b0 + 64, pr, tcn * 128:(tcn + 1) * 128],
                                        qt_sb[b0:b0 + 64, pr, sc * 512:(sc + 1) * 512],
                                        start=True, stop=True,
                                    )
                                if tcn == 0:
                                    flush_deferred()
                                    while pending:
                                        emit_norm()
                                nc.scalar.activation(
                                    pt2[:, sl, :], pst[:], AF.Exp,
                                    bias=gcol[:, tcn:tcn + 1], scale=0.125,
                                )
                                if sc == 0:
                                    # stagger remaining projections through
                                    # the sc=0 streams
                                    if pr == 0:
                                        if tcn == 1:
                                            qk_half(4, 1)
                                        elif tcn == 5:
                                            qk_half(5, 0)
                                        elif tcn == 6:
                                            qk_half(1, 0)
                                        elif tcn == 7:
                                            qk_half(5, 1)
                                    elif pr < 3:
                                        if tcn == 1:
                                            qk_half(4 + pr + 1, 0)
                                        elif tcn == 2:
                                            qk_half(pr + 1, 0)
                                        elif tcn == 7:
                                            qk_half(4 + pr + 1, 1)
                                    else:
                                        if tcn == 3:
                                            qk_half(0, 1)
                                        elif tcn >= 5:
                                            qk_half(tcn - 4, 1)
                                if sc == 1 and pr == 0 and tcn in (3, 5, 7):
                                    y_chunk((tcn - 3) // 2, 0)
                                elif sc == 1 and pr == 1 and tcn == 1:
                                    y_chunk(3, 0)

                            def mk_pv(j, pt2, pvp, pr, sc, sg):
                                def pv():
                                    osts = []
                                    for hi in range(2):
                                        nc.tensor.matmul(
                                            pvp[hi][0:66, :],
                                            v8[:, j, :, 2 * pr + hi, 0:66],
                                            pt2[:, :, hi * 512:(hi + 1) * 512],
                                            start=(j == 0), stop=(j == 3),
                                            perf_mode=PM.DoubleRow,
                                        )
                                        if j == 3:
                                            # drain: sigma recip (PSUM row 64)
                                            # + d rows to SBUF (frees psC)
                                            nc.vector.reciprocal(
                                                sg[64:65, hi * 512:(hi + 1) * 512],
                                                pvp[hi][64:65, 0:512])
                                            ost = ost_pool.tile([64, 512], bf16)
                                            nc.vector.tensor_copy(
                                                ost[:], pvp[hi][0:64, :])
                                            osts.append(ost)
                                    if j == 3:
                                        pending.append((pr, sc, sg, osts))
                                return pv

                            if sl == 1:
                                if j < 3:
                                    mk_pv(j, pt2, pvp, pr, sc, sg)()
                                else:
                                    deferred.append(mk_pv(j, pt2, pvp, pr, sc, sg))
                flush_deferred()
                while pending:
                    emit_norm(tail=True)
                for c in range(4):
                    y_chunk(c, 1)
                return out_tiles

            def src0(kc, sc):
                return xsb[:, kc, sc * 512:(sc + 1) * 512]

            def mk(t):
                def srcn(kc, sc):
                    return t[:, kc, sc * 512:(sc + 1) * 512]
                return srcn

            if repeat == 1:
                body(src0, y_d, 0)
            else:
                cur = src0
                for it in range(repeat):
                    nxt = None
                    if it < repeat - 1:
                        nxt = xpool.tile([128, 4, S], f32r, tag="xping",
                                         name=f"xping{it}")
                    body(cur, y_d, it, nxt)
                    if nxt is not None:
                        cur = mk(nxt)

    nc.finalize()
    return nc


def _host_prep(Wq, bq, Wk, bk, Wv, bv, Wo, bo, gamma, beta):
    import ml_dtypes

    g = np.asarray(gamma, np.float64)
    be = np.asarray(beta, np.float64)

    def eff(W, b):
        W = np.asarray(W, np.float64)
        b = np.asarray(b, np.float64)
        Wp = W * g[None, :]
        bp = b + W @ be
        return Wp, bp

    Wqp, bqp = eff(Wq, bq)
    Wkp, bkp = eff(Wk, bk)
    Wvp, bvp = eff(Wv, bv)
    f32 = np.float32
    E4 = ml_dtypes.float8_e4m3

    def to_dr(W):
        # W [C(=512) contraction, M out] -> [p(128), g(2), o(2), M] fp8,
        # channel c = (2g+o)*128 + p
        W = np.asarray(W, f32)
        M = W.shape[1]
        return np.ascontiguousarray(
            W.reshape(2, 2, 128, M).transpose(2, 0, 1, 3)
        ).astype(E4)

    wqk = to_dr(np.concatenate([Wqp.T, Wkp.T], axis=1))
    wv = to_dr(Wvp.T)
    # Wo^T in [d(64), pair(4), slot(2), out] order: channel c = pair*128 +
    # slot*64 + d matches the PV/norm partition-aligned layout
    woT = np.asarray(Wo, np.float64).T.astype(f32)
    wo = np.ascontiguousarray(
        woT.reshape(4, 2, 64, C).transpose(2, 0, 1, 3)
    ).astype(E4)

    u = Wkp.T @ bqp
    ucol = u.reshape(4, 128).T.astype(f32)
    nc0 = np.float32(-(bqp @ Wkp.sum(1)))
    bo_eff = np.asarray(bo, np.float64) + np.asarray(Wo, np.float64) @ bvp
    bocol = bo_eff.reshape(4, 128).T.astype(f32)
    csts = np.zeros((128, 9), f32)
    csts[:, 0:4] = ucol
    csts[:, 4:8] = bocol
    csts[0, 8] = nc0
    return dict(wqk=wqk, wv=wv, wo=wo, csts=csts)


def get_nc(repeat=1, debug=False):
    key = (repeat, debug)
    if key not in _CACHE:
        _CACHE[key] = _build_nc(repeat, debug)
    return _CACHE[key]


def make_in_maps(inputs):
    shared = _host_prep(
        inputs["Wq"], inputs["bq"], inputs["Wk"], inputs["bk"],
        inputs["Wv"], inputs["bv"], inputs["Wo"], inputs["bo"],
        inputs["gamma"], inputs["beta"],
    )
    x = np.asarray(inputs["x"], np.float32)
    in_maps = []
    for b in range(N_CORES):
        m = dict(shared)
        m["x"] = np.ascontiguousarray(x[b].reshape(C, S))
        in_maps.append(m)
    return in_maps


def kernel(**inputs):
    from concourse.bass_utils import run_bass_kernel_spmd

    nc = get_nc(repeat=1)
    in_maps = make_in_maps(inputs)
    res = run_bass_kernel_spmd(nc, in_maps, list(range(N_CORES)))
    out = np.stack([res.results[b]["y"].reshape(C, 32, 32) for b in range(N_CORES)])
    return out.astype(np.float32)


# revision 47
# speedup vs baseline: 1.1827x; 1.1827x over previous
"""Trainium2 Bass kernel for MultiHeadSelfAttention (B=8, C=512, H=W=32, 8 heads).

Sharding: data-parallel - one batch element per NeuronCore (8 cores).

v3: Act-engine-bound design. The 64 exp activations ([128,1024] each) are the
hard floor (~64us busy); everything else is cut or overlapped under them:
  - Q/K/V/out projections in fp8-e4m3 DoubleRow on a pre-centered (x - mu)
    fp8 copy (no rank-1 mu-fold matmuls; mu broadcast via one PE matmul,
    centering on the Pool engine).
  - PV in fp8 DoubleRow: exp writes fp8 probs into [128, 2, S] token-chunk
    pair tiles; v stored fp8 as [p, pair, slot, head, 66] with sigma-aug
    columns 64/65. pt carries 2^-k (k = -exponent(rsqrt(var)), a power of two
    exact in fp8) so the aug column (2^k) cancels exactly in the softmax
    denominator; v8 carries the residual r*2^k scale.
  - LayerNorm r = rsqrt(var) computed WITHOUT the Act engine: variance rows
    are PE-transposed to [128,4] columns, then a quake-style bit seed + two
    Newton steps run as tiny DVE ops; 2^k/k come from the exponent bits.
    The Act engine therefore runs Exp only - zero activation-table reloads
    and the exp stream is never interrupted.
  - All stats transposes are tiny PE is_transpose matmuls; no DRAM scratch.
  - Head pair-slot layout [64 d, pair, slot] keeps PV output, sigma
    normalization and the output projection partition-aligned: no
    SBUF->SBUF partition-move DMAs. Sigma rides PV rows 64/65, reciprocated
    on DVE and broadcast by one tiny PE matmul per head.
  - The last pair group's PV + drain are deferred into the next group's
    first score slot so the exp stream never waits on PV/normalization.
  - gpsimd DMAs are SWDGE (burn Pool-engine time): only the y stores ride
    the gpsimd queue; x/weight loads use the SP/Act hardware DGE queues,
    ordered x-half0, x-half1, wqk, wv, wo for the serial HBM stream.
  - In chained (repeat) mode each y chunk is copied into the next body's
    input tile as soon as it exists, so the next layer's stats overlap this
    layer's second-half attention (the body boundary costs ~0 Act idle).
"""

import math

import numpy as np

C = 512
S = 1024
B = 8
NH = 8
HD = 64
N_CORES = 8

LN2 = float(math.log(2.0))
MAGIC = float(2.0 ** 23)
QMAGIC = 0x5F3759DF

_CACHE = {}


def _build_nc(repeat=1, debug=False):
    import concourse.bass as bass
    import concourse.mybir as mybir
    import concourse.tile as tile
    from concourse import bacc, masks

    f32 = mybir.dt.float32
    f32r = mybir.dt.float32r
    bf16 = mybir.dt.bfloat16
    fp8 = mybir.dt.float8e4
    i32 = mybir.dt.int32
    AF = mybir.ActivationFunctionType
    OP = mybir.AluOpType
    PM = mybir.MatmulPerfMode

    nc = bacc.Bacc("TRN2", debug=False, num_devices=N_CORES)

    x_d = nc.declare_dram_parameter("x", [C, S], f32r, isOutput=False)
    wqk_d = nc.declare_dram_parameter("wqk", [128, 2, 2, 2 * C], fp8, isOutput=False)
    wv_d = nc.declare_dram_parameter("wv", [128, 2, 2, C], fp8, isOutput=False)
    wo_d = nc.declare_dram_parameter("wo", [64, 4, 2, C], fp8, isOutput=False)
    csts_d = nc.declare_dram_parameter("csts", [128, 9], f32r, isOutput=False)
    y_d = nc.declare_dram_parameter("y", [C, S], f32, isOutput=True)

    def r32(ap):
        return ap.bitcast(f32r)

    with tile.TileContext(nc) as tc:
        import contextlib

        with contextlib.ExitStack() as ctx:
            ctx.enter_context(nc.allow_low_precision(reason="fp8/bf16 attention"))
            const = ctx.enter_context(tc.tile_pool(name="const", bufs=1))
            big = ctx.enter_context(tc.tile_pool(name="big", bufs=1))
            xsq_pool = ctx.enter_context(tc.tile_pool(name="xsq", bufs=6))
            pt_pool = ctx.enter_context(tc.tile_pool(name="pt", bufs=6))
            ost_pool = ctx.enter_context(tc.tile_pool(name="ost", bufs=6))
            y_pool = ctx.enter_context(tc.tile_pool(name="ysb", bufs=4 if repeat == 1 else 8))
            stats_sb = ctx.enter_context(tc.tile_pool(name="stats_sb", bufs=1))
            sg_pool = ctx.enter_context(tc.tile_pool(name="sg", bufs=2))
            ps = ctx.enter_context(tc.tile_pool(name="ps", bufs=2, space="PSUM"))

            # ---- static loads (x first, chunked, so stats start early) --------
            xpool = ctx.enter_context(tc.tile_pool(name="xpool", bufs=1 if repeat == 1 else 2))
            xsb = xpool.tile([128, 4, S], f32r, tag="xping")
            x_re = x_d[:, :].rearrange("(kc p) s -> p kc s", p=128)
            wqk_sb = const.tile([128, 2, 2, 2 * C], fp8)
            wv_sb = const.tile([128, 2, 2, C], fp8)
            wo_sb = const.tile([64, 4, 2, C], fp8)
            csts_sb = const.tile([128, 9], f32r)

            def xdma(eng, kc, sc):
                eng.dma_start(
                    out=xsb[:, kc, sc * 512:(sc + 1) * 512],
                    in_=x_re[:, kc, sc * 512:(sc + 1) * 512])

            # Issue order tuned for the serial HBM stream: x half 0 first,
            # then k/q weights, then x half 1, then wv/wo. gpsimd DMAs are
            # SWDGE (cost Pool-engine time) so only 2 x chunks ride there.
            nc.scalar.dma_start(out=csts_sb[:], in_=csts_d[:, :])
            xdma(nc.sync, 0, 0)
            xdma(nc.scalar, 1, 0)
            xdma(nc.scalar, 2, 0)
            xdma(nc.sync, 3, 0)
            xdma(nc.scalar, 1, 1)
            xdma(nc.scalar, 2, 1)
            xdma(nc.sync, 0, 1)
            xdma(nc.sync, 3, 1)
            nc.scalar.dma_start(out=wqk_sb[:, :, :, 512:], in_=wqk_d[:, :, :, 512:])
            nc.sync.dma_start(out=wqk_sb[:, :, :, 0:512], in_=wqk_d[:, :, :, 0:512])
            nc.scalar.dma_start(out=wv_sb[:], in_=wv_d[:, :, :, :])
            nc.sync.dma_start(out=wo_sb[:], in_=wo_d[:, :, :, :])

            # PE p-state warm-up fuel: memset early (on DVE; Pool may be
            # busy with SWDGE x transfers) so the warm chain starts at t~1us
            warmmov = const.tile([128, 512], bf16)
            nc.vector.memset(warmmov[:], 0.0)
            onesb = const.tile([128, 1], bf16)
            nc.vector.memset(onesb[:], 1.0)
            ones1f = const.tile([1, 128], f32)
            nc.vector.memset(ones1f[:], 1.0)
            ones1 = const.tile([1, 128], f32r)
            nc.vector.tensor_copy(ones1[:], ones1f[:])
            onescf = const.tile([128, 1], f32)
            nc.vector.memset(onescf[:], 1.0)
            onesc = const.tile([128, 1], f32r)
            nc.vector.tensor_copy(onesc[:], onescf[:])
            # row of ones at partition 64: stationary for the sigma broadcast
            e64f = const.tile([65, 64], f32)
            nc.gpsimd.memset(e64f[64:65, :], 1.0)
            e64 = const.tile([65, 64], f32r)
            nc.vector.tensor_copy(e64[64:65, :], e64f[64:65, :])
            # 1x1 identity for PE row->column transposes
            ident1 = const.tile([1, 1], f32)
            nc.gpsimd.memset(ident1[:], 1.0)
            # 128x128 identity for column->row transposes
            idN = const.tile([128, 128], f32)
            masks.make_identity(nc, idN[:])
            # aug multiplier: writes the sigma columns 64,65 of v8
            augf2 = const.tile([128, 8, 2], f32)
            nc.vector.memset(augf2[:], 1.0)
            epsr = const.tile([1, 1], f32)
            nc.vector.memset(epsr[:], 1e-5)
            # prime the ln/exp table during idle startup
            warm = const.tile([1, 1], f32)
            nc.scalar.activation(warm[:], ones1f[0:1, 0:1], AF.Exp,
                                 bias=0.0, scale=1.0)
            # PE p-state warm-up: keep the tensor engine continuously busy
            # during the x load so the stats chain runs at full clock

            def psA(name):
                return ps.tile([128, S], f32, tag="psA", name=name)

            def psB(name):
                return ps.tile([128, 512], f32, tag="psB", name=name)

            def psBt(shape, name):
                return ps.tile(shape, f32, tag="psB", name=name)

            def psC(name):
                return ps.tile([128, 512], f32, tag="psC", name=name)

            warm_ps = ps.tile([128, S], f32, tag="psA", name="warm_ps")
            for w in range(4):
                nc.tensor.matmul(warm_ps[0:1, 0:512], onesb[:], warmmov[:],
                                 start=(w == 0), stop=(w == 3))

            def body(src, dst_dram, it, nxt=None):
                """One attention layer: src(kc, sc) -> writes dst_dram.
                When chaining (nxt given), each y chunk is copied into nxt
                as soon as it exists so the next body's stats can start
                while this body's second-half attention still runs."""
                x8 = big.tile([128, 2, 2, S], fp8, tag="x8", bufs=2)
                murow = stats_sb.tile([1, S], f32r, tag="murow")
                srowA = stats_sb.tile([1, S], f32, tag="srowA")
                srowB = stats_sb.tile([1, S], f32, tag="srowB")
                rrowS = stats_sb.tile([1, S], f32r, tag="rrowS")
                g1row = stats_sb.tile([1, S], f32, tag="g1row")
                kf = stats_sb.tile([128, 8], f32, tag="kf")
                sqcol = stats_sb.tile([128, 8], f32, tag="sqcol")
                rcol2 = stats_sb.tile([128, 8], f32, tag="rcol2")
                scol = stats_sb.tile([128, 8], f32, tag="scol")
                gt = stats_sb.tile([128, 8], f32, tag="gt")
                gcol = stats_sb.tile([128, 8], f32, tag="gcol")
                varE = stats_sb.tile([128, 8], f32, tag="varE")
                nw1 = stats_sb.tile([128, 8], f32, tag="nw1")
                nw2 = stats_sb.tile([128, 8], f32, tag="nw2")
                nw3 = stats_sb.tile([128, 8], f32, tag="nw3")
                R_sb = big.tile([128, S], f32, tag="R", bufs=2)
                M_sb = big.tile([128, S], f32, tag="M", bufs=2)

                # ---- per-token-half stats, staged so half 1 can be emitted
                # interleaved into the early attention stream (its x lands
                # late; emitting it upfront would block the in-order PE/DVE
                # queues behind the DMA wait) -------------------------------
                stq_rows = {}

                def stats_stage1(sc):
                    """sums -> mu -> MU broadcast -> x8 centering."""
                    h = slice(sc * 512, (sc + 1) * 512)
                    sts_t = psB(f"stx{it}_{sc}")
                    stq_t = psB(f"stq{it}_{sc}")
                    sts = sts_t[0:1, :]
                    for kc in range(4):
                        nc.tensor.matmul(
                            sts, r32(onesc[:]), r32(src(kc, sc)),
                            start=(kc == 0), stop=(kc == 3))
                    xsqs = []
                    for kc in range(4):
                        xs = src(kc, sc)
                        xsq = xsq_pool.tile([128, 512], f32r)
                        if kc < 2:
                            nc.gpsimd.tensor_mul(xsq[:], xs.bitcast(f32),
                                                 xs.bitcast(f32))
                        xsqs.append(xsq)
                    nc.vector.tensor_scalar_mul(murow[:, h], sts, 1.0 / C)
                    for kc in range(2, 4):
                        xs = src(kc, sc)
                        nc.gpsimd.tensor_mul(xsqs[kc][:], xs.bitcast(f32),
                                             xs.bitcast(f32))
                    ps_m = psB(f"ps_m{it}_{sc}")
                    nc.tensor.matmul(ps_m[:], r32(ones1[:]), r32(murow[:, h]),
                                     start=True, stop=True)
                    nc.vector.tensor_copy(M_sb[:, h], ps_m[:])
                    stq = stq_t[0:1, :]
                    for kc in range(4):
                        nc.tensor.matmul(
                            stq, r32(onesc[:]), r32(xsqs[kc][:]),
                            start=(kc == 0), stop=(kc == 3))
                    stq_rows[sc] = stq
                    for kc in range(4):
                        nc.gpsimd.tensor_tensor(
                            x8[:, kc // 2, kc % 2, h], src(kc, sc).bitcast(f32),
                            M_sb[:, h], OP.subtract)

                def stats_stage2(sc):
                    """variance -> transposed rsqrt (quake seed + 2 Newton
                    steps, all DVE bit ops; the Act engine stays Exp-only,
                    so the big-exp stream is never interrupted by table
                    loads) -> r row via back-transpose -> R broadcast."""
                    h = slice(sc * 512, (sc + 1) * 512)
                    cs = slice(4 * sc, 4 * sc + 4)
                    stq = stq_rows[sc]
                    nc.vector.tensor_mul(srowB[:, h], murow[:, h].bitcast(f32),
                                         murow[:, h].bitcast(f32))
                    nc.vector.scalar_tensor_tensor(
                        srowA[:, h], stq, 1.0 / C, srowB[:, h],
                        OP.mult, OP.subtract)
                    psT = psBt([128, 4, 2], f"psT{it}_{sc}")
                    for c in range(4):
                        c0 = sc * 512 + c * 128
                        nc.tensor.transpose(
                            psT[:, c, 0:1], srowA[0:1, c0:c0 + 128], ident1[:])
                    nc.vector.tensor_scalar_add(varE[:, cs], psT[:, :, 0], 1e-5)
                    vi = varE[:, cs].bitcast(i32)
                    nc.vector.tensor_scalar(nw1[:, cs].bitcast(i32), vi,
                                            1, None, OP.arith_shift_right)
                    # y0 bits = QMAGIC - (v>>1)
                    nc.vector.tensor_scalar(nw2[:, cs].bitcast(i32),
                                            nw1[:, cs].bitcast(i32),
                                            -1, QMAGIC, OP.mult, OP.add)
                    y0 = nw2[:, cs]
                    nc.vector.tensor_mul(nw3[:, cs], y0, y0)
                    nc.vector.tensor_mul(nw1[:, cs], nw3[:, cs], varE[:, cs])
                    nc.vector.tensor_scalar(nw3[:, cs], nw1[:, cs], -0.5, 1.5,
                                            OP.mult, OP.add)
                    nc.vector.tensor_mul(nw1[:, cs], y0, nw3[:, cs])
                    nc.vector.tensor_mul(nw2[:, cs], nw1[:, cs], nw1[:, cs])
                    nc.vector.tensor_mul(nw3[:, cs], nw2[:, cs], varE[:, cs])
                    nc.vector.tensor_scalar(nw2[:, cs], nw3[:, cs], -0.5, 1.5,
                                            OP.mult, OP.add)
                    nc.vector.tensor_mul(rcol2[:, cs], nw1[:, cs], nw2[:, cs])
                    psRow = psBt([1, 512], f"psRw{it}_{sc}")
                    for c in range(4):
                        nc.tensor.transpose(
                            psRow[0:1, c * 128:(c + 1) * 128],
                            rcol2[:, 4 * sc + c:4 * sc + c + 1], idN[:])
                    nc.vector.tensor_copy(rrowS[:, h], psRow[:])
                    ps_r = psB(f"ps_r{it}_{sc}")
                    nc.tensor.matmul(ps_r[:], r32(ones1[:]), rrowS[:, h],
                                     start=True, stop=True)
                    nc.vector.tensor_copy(R_sb[:, h], ps_r[:])

                def stats_stage3(sc):
                    """g1 row + transposed exp-bias cols; 2^k from the rsqrt
                    exponent bits (exact in fp8, cancels in sigma)."""
                    h = slice(sc * 512, (sc + 1) * 512)
                    cs = slice(4 * sc, 4 * sc + 4)
                    gps_t = psB(f"g{it}_{sc}")
                    gps = gps_t[0:1, :]
                    for kc in range(4):
                        nc.tensor.matmul(
                            gps, csts_sb[:, kc:kc + 1], r32(src(kc, sc)),
                            start=(kc == 0), stop=(kc == 3))
                    nc.vector.scalar_tensor_tensor(
                        g1row[:, h], murow[:, h].bitcast(f32), csts_sb[0:1, 8:9].bitcast(f32), gps,
                        OP.mult, OP.add)
                    psT = psBt([128, 4, 2], f"psTg{it}_{sc}")
                    for c in range(4):
                        c0 = sc * 512 + c * 128
                        nc.tensor.transpose(
                            psT[:, c, 1:2], g1row[0:1, c0:c0 + 128], ident1[:])
                    # k = -exponent(rcol2): 2^k and k exact from the bits
                    ri = rcol2[:, cs].bitcast(i32)
                    nc.vector.tensor_scalar(nw1[:, cs].bitcast(i32), ri,
                                            23, None, OP.arith_shift_right)
                    nc.vector.tensor_scalar(nw2[:, cs].bitcast(i32),
                                            nw1[:, cs].bitcast(i32),
                                            -1, 127, OP.mult, OP.add)
                    nc.vector.tensor_copy(kf[:, cs], nw2[:, cs].bitcast(i32))
                    nc.vector.tensor_scalar(nw3[:, cs].bitcast(i32),
                                            nw1[:, cs].bitcast(i32),
                                            -1, 254, OP.mult, OP.add)
                    nc.vector.tensor_scalar(sqcol[:, cs].bitcast(i32),
                                            nw3[:, cs].bitcast(i32),
                                            23, None, OP.arith_shift_left)
                    # v scale = r * 2^k (exact-cancel form of aug)
                    nc.vector.tensor_mul(scol[:, cs], sqcol[:, cs], rcol2[:, cs])
                    # exp bias: gcol = -k*ln2 + 0.125*r*g1
                    nc.vector.scalar_tensor_tensor(
                        gt[:, cs], psT[:, :, 1], 0.125, rcol2[:, cs],
                        OP.mult, OP.mult)
                    nc.vector.scalar_tensor_tensor(
                        gcol[:, cs], kf[:, cs], -LN2, gt[:, cs],
                        OP.mult, OP.add)

                stats_stage1(0)
                stats_stage2(0)
                stats_stage3(0)
                stats_stage1(1)
                stats_stage2(1)
                stats_stage3(1)

                # ---- QK projections (fp8 DoubleRow on centered x8) ------------
                qt_sb = big.tile([128, 4, S], bf16, tag="qt", bufs=2)
                kt_sb = big.tile([128, 4, S], bf16, tag="kt", bufs=2)

                def qk_half(oc, sc, pool=None):
                    dst = qt_sb if oc < 4 else kt_sb
                    o4 = oc % 4
                    h = slice(sc * 512, (sc + 1) * 512)
                    p = (pool or psB)(f"qk{it}_{oc}_{sc}")
                    for g in range(2):
                        nc.tensor.matmul(
                            p[:],
                            wqk_sb[:, g, :, oc * 128:(oc + 1) * 128],
                            x8[:, g, :, h],
                            start=(g == 0), stop=(g == 1),
                            perf_mode=PM.DoubleRow,
                        )
                    nc.vector.tensor_tensor(
                        dst[:, o4, h], p[:], R_sb[:, h], OP.mult)

                qk_half(4, 0, pool=psC)
                qk_half(0, 0)

                # v8[p_tok, pair, slot, head, 0:64] = fp8(v_raw * r * 2^k)
                # cols 64,65 = 2^k (sigma aug, exact in fp8; 66-wide keeps
                # the stationary AP run even for the ldweights ISA check)
                v8 = big.tile([128, 4, 2, 8, 66], fp8, tag="v8", bufs=2)

                def v_chunk(tcn):
                    j, sl = tcn // 2, tcn % 2
                    p = psB(f"v{it}_{tcn}")
                    for g in range(2):
                        nc.tensor.matmul(
                            p[:],
                            x8[:, g, :, tcn * 128:(tcn + 1) * 128],
                            wv_sb[:, g, :, :],
                            start=(g == 0), stop=(g == 1),
                            perf_mode=PM.DoubleRow,
                        )
                    nc.vector.tensor_scalar_mul(
                        v8[:, j, sl, :, 0:64],
                        p[:].rearrange("p (h d) -> p h d", h=8),
                        scol[:, tcn:tcn + 1])
                    nc.gpsimd.tensor_scalar_mul(
                        v8[:, j, sl, :, 64:66], augf2[:], sqcol[:, tcn:tcn + 1])

                # ---- attention: sc-major; per (sc, pr): 4 pair groups ---------
                # opk8[d, pair(pr), slot(hi), s]: normalized attn out, fp8
                opk8 = big.tile([64, 4, 2, S], fp8, tag="opk8", bufs=2)
                pending = []  # (pr, sc, sg, osts)

                def emit_norm(tail=False):
                    pr, sc, sg, osts = pending.pop(0)
                    h = slice(sc * 512, (sc + 1) * 512)
                    for hi in range(2):
                        ps_e = psB(f"pe{it}_{pr}_{sc}_{hi}")
                        nc.tensor.matmul(
                            ps_e[0:64, :],
                            e64[64:65, :],
                            sg[64:65, hi * 512:(hi + 1) * 512],
                            start=True, stop=True,
                        )
                        nc.vector.tensor_tensor(
                            opk8[:, pr, hi, h], osts[hi][:],
                            ps_e[0:64, :], OP.mult)

                def y_chunk(c, sc):
                    h = slice(sc * 512, (sc + 1) * 512)
                    ps_y = psB(f"y{it}_{c}_{sc}")
                    for j in range(4):
                        nc.tensor.matmul(
                            ps_y[:],
                            wo_sb[:, j, :, c * 128:(c + 1) * 128],
                            opk8[:, j, :, h],
                            start=(j == 0), stop=(j == 3),
                            perf_mode=PM.DoubleRow,
                        )
                    ysb = y_pool.tile([128, 512], f32)
                    nc.vector.scalar_tensor_tensor(
                        ysb[:], ps_y[:], csts_sb[:, 4 + c:5 + c].bitcast(f32),
                        src(c, sc).bitcast(f32),
                        OP.add, OP.add,
                    )
                    qeng = [nc.sync, nc.gpsimd, nc.sync, nc.gpsimd][c]
                    qeng.dma_start(
                        out=dst_dram[c * 128:(c + 1) * 128, h],
                        in_=ysb[:],
                    )
                    if nxt is not None:
                        nc.gpsimd.tensor_copy(nxt[:, c, h], ysb[:])
                    out_tiles.append(ysb)

                # deferred last-pair PV: emitted just after the next group's
                # first score matmuls so the Act exp stream never waits
                deferred = []

                def flush_deferred():
                    while deferred:
                        deferred.pop(0)()

                out_tiles = []
                for sc in range(2):
                    for pr in range(4):
                        sg = sg_pool.tile([65, S], f32r, tag="sg",
                                          name=f"sg{it}_{pr}_{sc}")
                        pvp = [psC(f"pv{it}_{pr}_{sc}_{hi}") for hi in range(2)]
                        for j in range(4):
                            pt2 = pt_pool.tile([128, 2, S], fp8)
                            for sl in range(2):
                                tcn = 2 * j + sl
                                if pr == 0 and sc == 0:
                                    v_chunk(tcn)
                                pst = psA(f"sc{it}_{pr}_{sc}_{tcn}")
                                for hi in range(2):
                                    b0 = 64 * hi
                                    nc.tensor.matmul(
                                        pst[:, hi * 512:(hi + 1) * 512],
                                        kt_sb[b0:b0 + 64, pr, tcn * 128:(tcn + 1) * 128],
                                        qt_sb[b0:b0 + 64, pr, sc * 512:(sc + 1) * 512],
                                        start=True, stop=True,
                                    )
                                if tcn == 0:
                                    flush_deferred()
                                    while pending:
                                        emit_norm()
                                nc.scalar.activation(
                                    pt2[:, sl, :], pst[:], AF.Exp,
                                    bias=gcol[:, tcn:tcn + 1], scale=0.125,
                                )
                                if sc == 0:
                                    # stagger remaining projections through
                                    # the sc=0 streams
                                    if pr == 0:
                                        if tcn == 1:
                                            qk_half(4, 1)
                                        elif tcn == 5:
                                            qk_half(5, 0)
                                        elif tcn == 6:
                                            qk_half(1, 0)
                                        elif tcn == 7:
                                            qk_half(5, 1)
                                    elif pr < 3:
                                        if tcn == 1:
                                            qk_half(4 + pr + 1, 0)
                                        elif tcn == 2:
                                            qk_half(pr + 1, 0)
                                        elif tcn == 7:
                                            qk_half(4 + pr + 1, 1)
                                    else:
                                        if tcn == 3:
                                            qk_half(0, 1)
                                        elif tcn >= 5:
                                            qk_half(tcn - 4, 1)
                                if sc == 1 and pr == 0 and tcn in (3, 5, 7):
                                    y_chunk((tcn - 3) // 2, 0)
                                elif sc == 1 and pr == 1 and tcn == 1:
                                    y_chunk(3, 0)

                            def mk_pv(j, pt2, pvp, pr, sc, sg):
                                def pv():
                                    osts = []
                                    for hi in range(2):
                                        nc.tensor.matmul(
                                            pvp[hi][0:66, :],
                                            v8[:, j, :, 2 * pr + hi, 0:66],
                                            pt2[:, :, hi * 512:(hi + 1) * 512],
                                            start=(j == 0), stop=(j == 3),
                                            perf_mode=PM.DoubleRow,
                                        )
                                        if j == 3:
                                            # drain: sigma recip (PSUM row 64)
                                            # + d rows to SBUF (frees psC)
                                            nc.vector.reciprocal(
                                                sg[64:65, hi * 512:(hi + 1) * 512],
                                                pvp[hi][64:65, 0:512])
                                            ost = ost_pool.tile([64, 512], bf16)
                                            nc.vector.tensor_copy(
                                                ost[:], pvp[hi][0:64, :])
                                            osts.append(ost)
                                    if j == 3:
                                        pending.append((pr, sc, sg, osts))
                                return pv

                            if sl == 1:
                                if j < 3:
                                    mk_pv(j, pt2, pvp, pr, sc, sg)()
                                else:
                                    deferred.append(mk_pv(j, pt2, pvp, pr, sc, sg))
                flush_deferred()
                while pending:
                    emit_norm(tail=True)
                for c in range(4):
                    y_chunk(c, 1)
                return out_tiles

            def src0(kc, sc):
                return xsb[:, kc, sc * 512:(sc + 1) * 512]

            def mk(t):
                def srcn(kc, sc):
                    return t[:, kc, sc * 512:(sc + 1) * 512]
                return srcn

            if repeat == 1:
                body(src0, y_d, 0)
            else:
                cur = src0
                for it in range(repeat):
                    nxt = None
                    if it < repeat - 1:
                        nxt = xpool.tile([128, 4, S], f32r, tag="xping",
                                         name=f"xping{it}")
                    body(cur, y_d, it, nxt)
                    if nxt is not None:
                        cur = mk(nxt)

    nc.finalize()
    return nc


def _host_prep(Wq, bq, Wk, bk, Wv, bv, Wo, bo, gamma, beta):
    import ml_dtypes

    g = np.asarray(gamma, np.float64)
    be = np.asarray(beta, np.float64)

    def eff(W, b):
        W = np.asarray(W, np.float64)
        b = np.asarray(b, np.float64)
        Wp = W * g[None, :]
        bp = b + W @ be
        return Wp, bp

    Wqp, bqp = eff(Wq, bq)
    Wkp, bkp = eff(Wk, bk)
    Wvp, bvp = eff(Wv, bv)
    f32 = np.float32
    E4 = ml_dtypes.float8_e4m3

    def to_dr(W):
        # W [C(=512) contraction, M out] -> [p(128), g(2), o(2), M] fp8,
        # channel c = (2g+o)*128 + p
        W = np.asarray(W, f32)
        M = W.shape[1]
        return np.ascontiguousarray(
            W.reshape(2, 2, 128, M).transpose(2, 0, 1, 3)
        ).astype(E4)

    wqk = to_dr(np.concatenate([Wqp.T, Wkp.T], axis=1))
    wv = to_dr(Wvp.T)
    # Wo^T in [d(64), pair(4), slot(2), out] order: channel c = pair*128 +
    # slot*64 + d matches the PV/norm partition-aligned layout
    woT = np.asarray(Wo, np.float64).T.astype(f32)
    wo = np.ascontiguousarray(
        woT.reshape(4, 2, 64, C).transpose(2, 0, 1, 3)
    ).astype(E4)

    u = Wkp.T @ bqp
    ucol = u.reshape(4, 128).T.astype(f32)
    nc0 = np.float32(-(bqp @ Wkp.sum(1)))
    bo_eff = np.asarray(bo, np.float64) + np.asarray(Wo, np.float64) @ bvp
    bocol = bo_eff.reshape(4, 128).T.astype(f32)
    csts = np.zeros((128, 9), f32)
    csts[:, 0:4] = ucol
    csts[:, 4:8] = bocol
    csts[0, 8] = nc0
    return dict(wqk=wqk, wv=wv, wo=wo, csts=csts)


def get_nc(repeat=1, debug=False):
    key = (repeat, debug)
    if key not in _CACHE:
        _CACHE[key] = _build_nc(repeat, debug)
    return _CACHE[key]


def make_in_maps(inputs):
    shared = _host_prep(
        inputs["Wq"], inputs["bq"], inputs["Wk"], inputs["bk"],
        inputs["Wv"], inputs["bv"], inputs["Wo"], inputs["bo"],
        inputs["gamma"], inputs["beta"],
    )
    x = np.asarray(inputs["x"], np.float32)
    in_maps = []
    for b in range(N_CORES):
        m = dict(shared)
        m["x"] = np.ascontiguousarray(x[b].reshape(C, S))
        in_maps.append(m)
    return in_maps


def kernel(**inputs):
    from concourse.bass_utils import run_bass_kernel_spmd

    nc = get_nc(repeat=1)
    in_maps = make_in_maps(inputs)
    res = run_bass_kernel_spmd(nc, in_maps, list(range(N_CORES)))
    out = np.stack([res.results[b]["y"].reshape(C, 32, 32) for b in range(N_CORES)])
    return out.astype(np.float32)


# revision 49
# speedup vs baseline: 1.2312x; 1.0410x over previous
"""Trainium2 Bass kernel for MultiHeadSelfAttention (B=8, C=512, H=W=32, 8 heads).

Sharding: data-parallel - one batch element per NeuronCore (8 cores).

v3: Act-engine-bound design. The 64 exp activations ([128,1024] each) are the
hard floor (~64us busy); everything else is cut or overlapped under them:
  - Q/K/V/out projections in fp8-e4m3 DoubleRow on a pre-centered (x - mu)
    fp8 copy (no rank-1 mu-fold matmuls; mu broadcast via one PE matmul,
    centering on the Pool engine).
  - PV in fp8 DoubleRow: exp writes fp8 probs into [128, 2, S] token-chunk
    pair tiles; v stored fp8 as [p, pair, slot, head, 66] with sigma-aug
    columns 64/65. pt carries 2^-k (k = -exponent(rsqrt(var)), a power of two
    exact in fp8) so the aug column (2^k) cancels exactly in the softmax
    denominator; v8 carries the residual r*2^k scale.
  - LayerNorm r = rsqrt(var) computed WITHOUT the Act engine: variance rows
    are PE-transposed to [128,4] columns, then a quake-style bit seed + two
    Newton steps run as tiny DVE ops; 2^k/k come from the exponent bits.
    The Act engine therefore runs Exp only - zero activation-table reloads
    and the exp stream is never interrupted.
  - All stats transposes are tiny PE is_transpose matmuls; no DRAM scratch.
  - Head pair-slot layout [64 d, pair, slot] keeps PV output, sigma
    normalization and the output projection partition-aligned: no
    SBUF->SBUF partition-move DMAs. Sigma rides PV rows 64/65, reciprocated
    on DVE and broadcast by one tiny PE matmul per head.
  - The last pair group's PV + drain are deferred into the next group's
    first score slot so the exp stream never waits on PV/normalization.
  - gpsimd DMAs are SWDGE (burn Pool-engine time): only the y stores ride
    the gpsimd queue; x/weight loads use the SP/Act hardware DGE queues,
    ordered x-half0, x-half1, wqk, wv, wo for the serial HBM stream.
  - In chained (repeat) mode each y chunk is copied into the next body's
    input tile as soon as it exists, so the next layer's stats overlap this
    layer's second-half attention (the body boundary costs ~0 Act idle).
"""

import math

import numpy as np

C = 512
S = 1024
B = 8
NH = 8
HD = 64
N_CORES = 8

LN2 = float(math.log(2.0))
MAGIC = float(2.0 ** 23)
QMAGIC = 0x5F3759DF

_CACHE = {}


def _build_nc(repeat=1, debug=False):
    import concourse.bass as bass
    import concourse.mybir as mybir
    import concourse.tile as tile
    from concourse import bacc, masks

    f32 = mybir.dt.float32
    f32r = mybir.dt.float32r
    bf16 = mybir.dt.bfloat16
    fp8 = mybir.dt.float8e4
    i32 = mybir.dt.int32
    AF = mybir.ActivationFunctionType
    OP = mybir.AluOpType
    PM = mybir.MatmulPerfMode

    nc = bacc.Bacc("TRN2", debug=False, num_devices=N_CORES)

    x_d = nc.declare_dram_parameter("x", [C, S], f32r, isOutput=False)
    wqk_d = nc.declare_dram_parameter("wqk", [128, 2, 2, 2 * C], fp8, isOutput=False)
    wv_d = nc.declare_dram_parameter("wv", [128, 2, 2, C], fp8, isOutput=False)
    wo_d = nc.declare_dram_parameter("wo", [64, 4, 2, C], fp8, isOutput=False)
    csts_d = nc.declare_dram_parameter("csts", [128, 9], f32r, isOutput=False)
    y_d = nc.declare_dram_parameter("y", [C, S], f32, isOutput=True)

    def r32(ap):
        return ap.bitcast(f32r)

    with tile.TileContext(nc) as tc:
        import contextlib

        with contextlib.ExitStack() as ctx:
            ctx.enter_context(nc.allow_low_precision(reason="fp8/bf16 attention"))
            const = ctx.enter_context(tc.tile_pool(name="const", bufs=1))
            big = ctx.enter_context(tc.tile_pool(name="big", bufs=1))
            xsq_pool = ctx.enter_context(tc.tile_pool(name="xsq", bufs=6))
            pt_pool = ctx.enter_context(tc.tile_pool(name="pt", bufs=6))
            ost_pool = ctx.enter_context(tc.tile_pool(name="ost", bufs=6))
            y_pool = ctx.enter_context(tc.tile_pool(name="ysb", bufs=4 if repeat == 1 else 8))
            stats_sb = ctx.enter_context(tc.tile_pool(name="stats_sb", bufs=1))
            sg_pool = ctx.enter_context(tc.tile_pool(name="sg", bufs=2))
            ps = ctx.enter_context(tc.tile_pool(name="ps", bufs=2, space="PSUM"))

            # ---- static loads (x first, chunked, so stats start early) --------
            xpool = ctx.enter_context(tc.tile_pool(name="xpool", bufs=1 if repeat == 1 else 2))
            xsb = xpool.tile([128, 4, S], f32r, tag="xping")
            x_re = x_d[:, :].rearrange("(kc p) s -> p kc s", p=128)
            wqk_sb = const.tile([128, 2, 2, 2 * C], fp8)
            wv_sb = const.tile([128, 2, 2, C], fp8)
            wo_sb = const.tile([64, 4, 2, C], fp8)
            csts_sb = const.tile([128, 9], f32r)

            def xdma(eng, kc, sc):
                eng.dma_start(
                    out=xsb[:, kc, sc * 512:(sc + 1) * 512],
                    in_=x_re[:, kc, sc * 512:(sc + 1) * 512])

            # Issue order tuned for the serial HBM stream: x half 0 first,
            # then k/q weights, then x half 1, then wv/wo. gpsimd DMAs are
            # SWDGE (cost Pool-engine time) so only 2 x chunks ride there.
            nc.scalar.dma_start(out=csts_sb[:], in_=csts_d[:, :])
            xdma(nc.sync, 0, 0)
            xdma(nc.scalar, 1, 0)
            xdma(nc.scalar, 2, 0)
            xdma(nc.sync, 3, 0)
            xdma(nc.scalar, 1, 1)
            xdma(nc.scalar, 2, 1)
            xdma(nc.sync, 0, 1)
            xdma(nc.sync, 3, 1)
            nc.scalar.dma_start(out=wqk_sb[:, :, :, 512:], in_=wqk_d[:, :, :, 512:])
            nc.sync.dma_start(out=wqk_sb[:, :, :, 0:512], in_=wqk_d[:, :, :, 0:512])
            nc.scalar.dma_start(out=wv_sb[:], in_=wv_d[:, :, :, :])
            nc.sync.dma_start(out=wo_sb[:], in_=wo_d[:, :, :, :])

            # PE p-state warm-up fuel: memset early (on DVE; Pool may be
            # busy with SWDGE x transfers) so the warm chain starts at t~1us
            warmmov = const.tile([128, 512], bf16)
            nc.vector.memset(warmmov[:], 0.0)
            onesb = const.tile([128, 1], bf16)
            nc.vector.memset(onesb[:], 1.0)
            ones1f = const.tile([1, 128], f32)
            nc.vector.memset(ones1f[:], 1.0)
            ones1 = const.tile([1, 128], f32r)
            nc.vector.tensor_copy(ones1[:], ones1f[:])
            onescf = const.tile([128, 1], f32)
            nc.vector.memset(onescf[:], 1.0)
            onesc = const.tile([128, 1], f32r)
            nc.vector.tensor_copy(onesc[:], onescf[:])
            # row of ones at partition 64: stationary for the sigma broadcast
            e64f = const.tile([65, 64], f32)
            nc.gpsimd.memset(e64f[64:65, :], 1.0)
            e64 = const.tile([65, 64], f32r)
            nc.vector.tensor_copy(e64[64:65, :], e64f[64:65, :])
            # 1x1 identity for PE row->column transposes
            ident1 = const.tile([1, 1], f32)
            nc.gpsimd.memset(ident1[:], 1.0)
            # 128x128 identity for column->row transposes
            idN = const.tile([128, 128], f32)
            masks.make_identity(nc, idN[:])
            # aug multiplier: writes the sigma columns 64,65 of v8
            augf2 = const.tile([128, 8, 2], f32)
            nc.vector.memset(augf2[:], 1.0)
            epsr = const.tile([1, 1], f32)
            nc.vector.memset(epsr[:], 1e-5)
            # prime the ln/exp table during idle startup
            warm = const.tile([1, 1], f32)
            nc.scalar.activation(warm[:], ones1f[0:1, 0:1], AF.Exp,
                                 bias=0.0, scale=1.0)
            # PE p-state warm-up: keep the tensor engine continuously busy
            # during the x load so the stats chain runs at full clock

            def psA(name):
                return ps.tile([128, S], f32, tag="psA", name=name)

            def psB(name):
                return ps.tile([128, 512], f32, tag="psB", name=name)

            def psBt(shape, name):
                return ps.tile(shape, f32, tag="psB", name=name)

            def psC(name):
                return ps.tile([128, 512], f32, tag="psC", name=name)

            warm_ps = ps.tile([128, S], f32, tag="psA", name="warm_ps")
            for w in range(4):
                nc.tensor.matmul(warm_ps[0:1, 0:512], onesb[:], warmmov[:],
                                 start=(w == 0), stop=(w == 3))

            def body(src, dst_dram, it, nxt=None):
                """One attention layer: src(kc, sc) -> writes dst_dram.
                When chaining (nxt given), each y chunk is copied into nxt
                as soon as it exists so the next body's stats can start
                while this body's second-half attention still runs."""
                x8 = big.tile([128, 2, 2, S], fp8, tag="x8", bufs=2)
                murow = stats_sb.tile([1, S], f32r, tag="murow")
                srowA = stats_sb.tile([1, S], f32, tag="srowA")
                srowB = stats_sb.tile([1, S], f32, tag="srowB")
                rrowS = stats_sb.tile([1, S], f32r, tag="rrowS")
                g1row = stats_sb.tile([1, S], f32, tag="g1row")
                kf = stats_sb.tile([128, 8], f32, tag="kf")
                sqcol = stats_sb.tile([128, 8], f32, tag="sqcol")
                rcol2 = stats_sb.tile([128, 8], f32, tag="rcol2")
                scol = stats_sb.tile([128, 8], f32, tag="scol")
                gt = stats_sb.tile([128, 8], f32, tag="gt")
                gcol = stats_sb.tile([128, 8], f32, tag="gcol")
                varE = stats_sb.tile([128, 8], f32, tag="varE")
                nw1 = stats_sb.tile([128, 8], f32, tag="nw1")
                nw2 = stats_sb.tile([128, 8], f32, tag="nw2")
                nw3 = stats_sb.tile([128, 8], f32, tag="nw3")
                R_sb = big.tile([128, S], f32, tag="R", bufs=2)
                M_sb = big.tile([128, S], f32, tag="M", bufs=2)

                # ---- per-token-half stats, staged so half 1 can be emitted
                # interleaved into the early attention stream (its x lands
                # late; emitting it upfront would block the in-order PE/DVE
                # queues behind the DMA wait) -------------------------------
                stq_rows = {}

                def stats_stage1(sc):
                    """sums -> mu -> MU broadcast -> x8 centering."""
                    h = slice(sc * 512, (sc + 1) * 512)
                    sts_t = psB(f"stx{it}_{sc}")
                    stq_t = psB(f"stq{it}_{sc}")
                    sts = sts_t[0:1, :]
                    for kc in range(4):
                        nc.tensor.matmul(
                            sts, r32(onesc[:]), r32(src(kc, sc)),
                            start=(kc == 0), stop=(kc == 3))
                    xsqs = []
                    for kc in range(4):
                        xs = src(kc, sc)
                        xsq = xsq_pool.tile([128, 512], f32r)
                        if kc < 2:
                            nc.gpsimd.tensor_mul(xsq[:], xs.bitcast(f32),
                                                 xs.bitcast(f32))
                        xsqs.append(xsq)
                    nc.vector.tensor_scalar_mul(murow[:, h], sts, 1.0 / C)
                    for kc in range(2, 4):
                        xs = src(kc, sc)
                        nc.gpsimd.tensor_mul(xsqs[kc][:], xs.bitcast(f32),
                                             xs.bitcast(f32))
                    ps_m = psB(f"ps_m{it}_{sc}")
                    nc.tensor.matmul(ps_m[:], r32(ones1[:]), r32(murow[:, h]),
                                     start=True, stop=True)
                    nc.vector.tensor_copy(M_sb[:, h], ps_m[:])
                    stq = stq_t[0:1, :]
                    for kc in range(4):
                        nc.tensor.matmul(
                            stq, r32(onesc[:]), r32(xsqs[kc][:]),
                            start=(kc == 0), stop=(kc == 3))
                    stq_rows[sc] = stq
                    for kc in range(4):
                        nc.gpsimd.tensor_tensor(
                            x8[:, kc // 2, kc % 2, h], src(kc, sc).bitcast(f32),
                            M_sb[:, h], OP.subtract)

                def stats_stage2(sc):
                    """variance -> transposed rsqrt (quake seed + 2 Newton
                    steps, all DVE bit ops; the Act engine stays Exp-only,
                    so the big-exp stream is never interrupted by table
                    loads) -> r row via back-transpose -> R broadcast."""
                    h = slice(sc * 512, (sc + 1) * 512)
                    cs = slice(4 * sc, 4 * sc + 4)
                    stq = stq_rows[sc]
                    nc.vector.tensor_mul(srowB[:, h], murow[:, h].bitcast(f32),
                                         murow[:, h].bitcast(f32))
                    nc.vector.scalar_tensor_tensor(
                        srowA[:, h], stq, 1.0 / C, srowB[:, h],
                        OP.mult, OP.subtract)
                    psT = psBt([128, 4, 2], f"psT{it}_{sc}")
                    for c in range(4):
                        c0 = sc * 512 + c * 128
                        nc.tensor.transpose(
                            psT[:, c, 0:1], srowA[0:1, c0:c0 + 128], ident1[:])
                    nc.vector.tensor_scalar_add(varE[:, cs], psT[:, :, 0], 1e-5)
                    vi = varE[:, cs].bitcast(i32)
                    nc.vector.tensor_scalar(nw1[:, cs].bitcast(i32), vi,
                                            1, None, OP.arith_shift_right)
                    # y0 bits = QMAGIC - (v>>1)
                    nc.vector.tensor_scalar(nw2[:, cs].bitcast(i32),
                                            nw1[:, cs].bitcast(i32),
                                            -1, QMAGIC, OP.mult, OP.add)
                    y0 = nw2[:, cs]
                    nc.vector.tensor_mul(nw3[:, cs], y0, y0)
                    nc.vector.tensor_mul(nw1[:, cs], nw3[:, cs], varE[:, cs])
                    nc.vector.tensor_scalar(nw3[:, cs], nw1[:, cs], -0.5, 1.5,
                                            OP.mult, OP.add)
                    nc.vector.tensor_mul(nw1[:, cs], y0, nw3[:, cs])
                    # second Newton step, interleavable (off the latency path
                    # of 2^k which only needs the exponent): refine into rcol2
                    nc.vector.tensor_mul(nw2[:, cs], nw1[:, cs], nw1[:, cs])
                    nc.vector.tensor_mul(nw3[:, cs], nw2[:, cs], varE[:, cs])
                    nc.vector.tensor_scalar(nw2[:, cs], nw3[:, cs], -0.5, 1.5,
                                            OP.mult, OP.add)
                    nc.vector.tensor_mul(rcol2[:, cs], nw1[:, cs], nw2[:, cs])
                    psRow = psBt([1, 512], f"psRw{it}_{sc}")
                    for c in range(4):
                        nc.tensor.transpose(
                            psRow[0:1, c * 128:(c + 1) * 128],
                            rcol2[:, 4 * sc + c:4 * sc + c + 1], idN[:])
                    nc.vector.tensor_copy(rrowS[:, h], psRow[:])
                    ps_r = psB(f"ps_r{it}_{sc}")
                    nc.tensor.matmul(ps_r[:], r32(ones1[:]), rrowS[:, h],
                                     start=True, stop=True)
                    nc.vector.tensor_copy(R_sb[:, h], ps_r[:])

                def stats_stage3(sc):
                    """g1 row + transposed exp-bias cols; 2^k from the rsqrt
                    exponent bits (exact in fp8, cancels in sigma)."""
                    h = slice(sc * 512, (sc + 1) * 512)
                    cs = slice(4 * sc, 4 * sc + 4)
                    gps_t = psB(f"g{it}_{sc}")
                    gps = gps_t[0:1, :]
                    for kc in range(4):
                        nc.tensor.matmul(
                            gps, csts_sb[:, kc:kc + 1], r32(src(kc, sc)),
                            start=(kc == 0), stop=(kc == 3))
                    nc.vector.scalar_tensor_tensor(
                        g1row[:, h], murow[:, h].bitcast(f32), csts_sb[0:1, 8:9].bitcast(f32), gps,
                        OP.mult, OP.add)
                    psT = psBt([128, 4, 2], f"psTg{it}_{sc}")
                    for c in range(4):
                        c0 = sc * 512 + c * 128
                        nc.tensor.transpose(
                            psT[:, c, 1:2], g1row[0:1, c0:c0 + 128], ident1[:])
                    # k = -exponent(rcol2): 2^k and k exact from the bits
                    ri = rcol2[:, cs].bitcast(i32)
                    nc.vector.tensor_scalar(nw1[:, cs].bitcast(i32), ri,
                                            23, None, OP.arith_shift_right)
                    nc.vector.tensor_scalar(nw2[:, cs].bitcast(i32),
                                            nw1[:, cs].bitcast(i32),
                                            -1, 127, OP.mult, OP.add)
                    nc.vector.tensor_copy(kf[:, cs], nw2[:, cs].bitcast(i32))
                    nc.vector.tensor_scalar(nw3[:, cs].bitcast(i32),
                                            nw1[:, cs].bitcast(i32),
                                            -1, 254, OP.mult, OP.add)
                    nc.vector.tensor_scalar(sqcol[:, cs].bitcast(i32),
                                            nw3[:, cs].bitcast(i32),
                                            23, None, OP.arith_shift_left)
                    # v scale = r * 2^k (exact-cancel form of aug)
                    nc.vector.tensor_mul(scol[:, cs], sqcol[:, cs], rcol2[:, cs])
                    # exp bias: gcol = -k*ln2 + 0.125*r*g1
                    nc.vector.scalar_tensor_tensor(
                        gt[:, cs], psT[:, :, 1], 0.125, rcol2[:, cs],
                        OP.mult, OP.mult)
                    nc.vector.scalar_tensor_tensor(
                        gcol[:, cs], kf[:, cs], -LN2, gt[:, cs],
                        OP.mult, OP.add)

                # ---- QK projections (fp8 DoubleRow on centered x8) ------------
                qt_sb = big.tile([128, 4, S], bf16, tag="qt", bufs=2)
                kt_sb = big.tile([128, 4, S], bf16, tag="kt", bufs=2)

                def qk_half(oc, sc, pool=None):
                    dst = qt_sb if oc < 4 else kt_sb
                    o4 = oc % 4
                    h = slice(sc * 512, (sc + 1) * 512)
                    p = (pool or psB)(f"qk{it}_{oc}_{sc}")
                    for g in range(2):
                        nc.tensor.matmul(
                            p[:],
                            wqk_sb[:, g, :, oc * 128:(oc + 1) * 128],
                            x8[:, g, :, h],
                            start=(g == 0), stop=(g == 1),
                            perf_mode=PM.DoubleRow,
                        )
                    nc.vector.tensor_tensor(
                        dst[:, o4, h], p[:], R_sb[:, h], OP.mult)

                # half-0 stats, then the first q/k projections BEFORE the
                # half-1 stats: the sc0 attention stream starts while the
                # half-1 chains still run on the other engines
                stats_stage1(0)
                stats_stage2(0)
                stats_stage3(0)
                qk_half(4, 0, pool=psC)
                qk_half(0, 0)
                stats_stage1(1)
                stats_stage2(1)
                stats_stage3(1)

                # v8[p_tok, pair, slot, head, 0:64] = fp8(v_raw * r * 2^k)
                # cols 64,65 = 2^k (sigma aug, exact in fp8; 66-wide keeps
                # the stationary AP run even for the ldweights ISA check)
                v8 = big.tile([128, 4, 2, 8, 66], fp8, tag="v8", bufs=2)

                def v_chunk(tcn):
                    j, sl = tcn // 2, tcn % 2
                    p = psB(f"v{it}_{tcn}")
                    for g in range(2):
                        nc.tensor.matmul(
                            p[:],
                            x8[:, g, :, tcn * 128:(tcn + 1) * 128],
                            wv_sb[:, g, :, :],
                            start=(g == 0), stop=(g == 1),
                            perf_mode=PM.DoubleRow,
                        )
                    nc.vector.tensor_scalar_mul(
                        v8[:, j, sl, :, 0:64],
                        p[:].rearrange("p (h d) -> p h d", h=8),
                        scol[:, tcn:tcn + 1])
                    nc.gpsimd.tensor_scalar_mul(
                        v8[:, j, sl, :, 64:66], augf2[:], sqcol[:, tcn:tcn + 1])

                # ---- attention: sc-major; per (sc, pr): 4 pair groups ---------
                # opk8[d, pair(pr), slot(hi), s]: normalized attn out, fp8
                opk8 = big.tile([64, 4, 2, S], fp8, tag="opk8", bufs=2)
                pending = []  # (pr, sc, sg, osts)

                def emit_norm(tail=False):
                    pr, sc, sg, osts = pending.pop(0)
                    h = slice(sc * 512, (sc + 1) * 512)
                    for hi in range(2):
                        ps_e = psB(f"pe{it}_{pr}_{sc}_{hi}")
                        nc.tensor.matmul(
                            ps_e[0:64, :],
                            e64[64:65, :],
                            sg[64:65, hi * 512:(hi + 1) * 512],
                            start=True, stop=True,
                        )
                        nc.vector.tensor_tensor(
                            opk8[:, pr, hi, h], osts[hi][:],
                            ps_e[0:64, :], OP.mult)

                def y_chunk(c, sc):
                    h = slice(sc * 512, (sc + 1) * 512)
                    ps_y = psB(f"y{it}_{c}_{sc}")
                    for j in range(4):
                        nc.tensor.matmul(
                            ps_y[:],
                            wo_sb[:, j, :, c * 128:(c + 1) * 128],
                            opk8[:, j, :, h],
                            start=(j == 0), stop=(j == 3),
                            perf_mode=PM.DoubleRow,
                        )
                    ysb = y_pool.tile([128, 512], f32)
                    nc.vector.scalar_tensor_tensor(
                        ysb[:], ps_y[:], csts_sb[:, 4 + c:5 + c].bitcast(f32),
                        src(c, sc).bitcast(f32),
                        OP.add, OP.add,
                    )
                    qeng = [nc.sync, nc.gpsimd, nc.sync, nc.gpsimd][c]
                    qeng.dma_start(
                        out=dst_dram[c * 128:(c + 1) * 128, h],
                        in_=ysb[:],
                    )
                    if nxt is not None:
                        nc.gpsimd.tensor_copy(nxt[:, c, h], ysb[:])
                    out_tiles.append(ysb)

                # deferred last-pair PV: emitted just after the next group's
                # first score matmuls so the Act exp stream never waits
                deferred = []

                def flush_deferred():
                    while deferred:
                        deferred.pop(0)()

                out_tiles = []
                for sc in range(2):
                    for pr in range(4):
                        sg = sg_pool.tile([65, S], f32r, tag="sg",
                                          name=f"sg{it}_{pr}_{sc}")
                        pvp = [psC(f"pv{it}_{pr}_{sc}_{hi}") for hi in range(2)]
                        for j in range(4):
                            pt2 = pt_pool.tile([128, 2, S], fp8)
                            for sl in range(2):
                                tcn = 2 * j + sl
                                if pr == 0 and sc == 0:
                                    v_chunk(tcn)
                                pst = psA(f"sc{it}_{pr}_{sc}_{tcn}")
                                for hi in range(2):
                                    b0 = 64 * hi
                                    nc.tensor.matmul(
                                        pst[:, hi * 512:(hi + 1) * 512],
                                        kt_sb[b0:b0 + 64, pr, tcn * 128:(tcn + 1) * 128],
                                        qt_sb[b0:b0 + 64, pr, sc * 512:(sc + 1) * 512],
                                        start=True, stop=True,
                                    )
                                if tcn == 0:
                                    flush_deferred()
                                    while pending:
                                        emit_norm()
                                nc.scalar.activation(
                                    pt2[:, sl, :], pst[:], AF.Exp,
                                    bias=gcol[:, tcn:tcn + 1], scale=0.125,
                                )
                                if sc == 0:
                                    # stagger remaining projections through
                                    # the sc=0 streams
                                    if pr == 0:
                                        if tcn == 1:
                                            qk_half(4, 1)
                                        elif tcn == 5:
                                            qk_half(5, 0)
                                        elif tcn == 6:
                                            qk_half(1, 0)
                                        elif tcn == 7:
                                            qk_half(5, 1)
                                    elif pr < 3:
                                        if tcn == 1:
                                            qk_half(4 + pr + 1, 0)
                                        elif tcn == 2:
                                            qk_half(pr + 1, 0)
                                        elif tcn == 7:
                                            qk_half(4 + pr + 1, 1)
                                    else:
                                        if tcn == 3:
                                            qk_half(0, 1)
                                        elif tcn >= 5:
                                            qk_half(tcn - 4, 1)
                                if sc == 1 and pr == 0 and tcn in (3, 5, 7):
                                    y_chunk((tcn - 3) // 2, 0)
                                elif sc == 1 and pr == 1 and tcn == 1:
                                    y_chunk(3, 0)

                            def mk_pv(j, pt2, pvp, pr, sc, sg):
                                def pv():
                                    osts = []
                                    for hi in range(2):
                                        nc.tensor.matmul(
                                            pvp[hi][0:66, :],
                                            v8[:, j, :, 2 * pr + hi, 0:66],
                                            pt2[:, :, hi * 512:(hi + 1) * 512],
                                            start=(j == 0), stop=(j == 3),
                                            perf_mode=PM.DoubleRow,
                                        )
                                        if j == 3:
                                            # drain: sigma recip (PSUM row 64)
                                            # + d rows to SBUF (frees psC)
                                            nc.vector.reciprocal(
                                                sg[64:65, hi * 512:(hi + 1) * 512],
                                                pvp[hi][64:65, 0:512])
                                            ost = ost_pool.tile([64, 512], bf16)
                                            nc.vector.tensor_copy(
                                                ost[:], pvp[hi][0:64, :])
                                            osts.append(ost)
                                    if j == 3:
                                        pending.append((pr, sc, sg, osts))
                                return pv

                            if sl == 1:
                                if j < 3:
                                    mk_pv(j, pt2, pvp, pr, sc, sg)()
                                else:
                                    deferred.append(mk_pv(j, pt2, pvp, pr, sc, sg))
                flush_deferred()
                while pending:
                    emit_norm(tail=True)
                for c in range(4):
                    y_chunk(c, 1)
                return out_tiles

            def src0(kc, sc):
                return xsb[:, kc, sc * 512:(sc + 1) * 512]

            def mk(t):
                def srcn(kc, sc):
                    return t[:, kc, sc * 512:(sc + 1) * 512]
                return srcn

            if repeat == 1:
                body(src0, y_d, 0)
            else:
                cur = src0
                for it in range(repeat):
                    nxt = None
                    if it < repeat - 1:
                        nxt = xpool.tile([128, 4, S], f32r, tag="xping",
                                         name=f"xping{it}")
                    body(cur, y_d, it, nxt)
                    if nxt is not None:
                        cur = mk(nxt)

    nc.finalize()
    return nc


def _host_prep(Wq, bq, Wk, bk, Wv, bv, Wo, bo, gamma, beta):
    import ml_dtypes

    g = np.asarray(gamma, np.float64)
    be = np.asarray(beta, np.float64)

    def eff(W, b):
        W = np.asarray(W, np.float64)
        b = np.asarray(b, np.float64)
        Wp = W * g[None, :]
        bp = b + W @ be
        return Wp, bp

    Wqp, bqp = eff(Wq, bq)
    Wkp, bkp = eff(Wk, bk)
    Wvp, bvp = eff(Wv, bv)
    f32 = np.float32
    E4 = ml_dtypes.float8_e4m3

    def to_dr(W):
        # W [C(=512) contraction, M out] -> [p(128), g(2), o(2), M] fp8,
        # channel c = (2g+o)*128 + p
        W = np.asarray(W, f32)
        M = W.shape[1]
        return np.ascontiguousarray(
            W.reshape(2, 2, 128, M).transpose(2, 0, 1, 3)
        ).astype(E4)

    wqk = to_dr(np.concatenate([Wqp.T, Wkp.T], axis=1))
    wv = to_dr(Wvp.T)
    # Wo^T in [d(64), pair(4), slot(2), out] order: channel c = pair*128 +
    # slot*64 + d matches the PV/norm partition-aligned layout
    woT = np.asarray(Wo, np.float64).T.astype(f32)
    wo = np.ascontiguousarray(
        woT.reshape(4, 2, 64, C).transpose(2, 0, 1, 3)
    ).astype(E4)

    u = Wkp.T @ bqp
    ucol = u.reshape(4, 128).T.astype(f32)
    nc0 = np.float32(-(bqp @ Wkp.sum(1)))
    bo_eff = np.asarray(bo, np.float64) + np.asarray(Wo, np.float64) @ bvp
    bocol = bo_eff.reshape(4, 128).T.astype(f32)
    csts = np.zeros((128, 9), f32)
    csts[:, 0:4] = ucol
    csts[:, 4:8] = bocol
    csts[0, 8] = nc0
    return dict(wqk=wqk, wv=wv, wo=wo, csts=csts)


def get_nc(repeat=1, debug=False):
    key = (repeat, debug)
    if key not in _CACHE:
        _CACHE[key] = _build_nc(repeat, debug)
    return _CACHE[key]


def make_in_maps(inputs):
    shared = _host_prep(
        inputs["Wq"], inputs["bq"], inputs["Wk"], inputs["bk"],
        inputs["Wv"], inputs["bv"], inputs["Wo"], inputs["bo"],
        inputs["gamma"], inputs["beta"],
    )
    x = np.asarray(inputs["x"], np.float32)
    in_maps = []
    for b in range(N_CORES):
        m = dict(shared)
        m["x"] = np.ascontiguousarray(x[b].reshape(C, S))
        in_maps.append(m)
    return in_maps


def kernel(**inputs):
    from concourse.bass_utils import run_bass_kernel_spmd

    nc = get_nc(repeat=1)
    in_maps = make_in_maps(inputs)
    res = run_bass_kernel_spmd(nc, in_maps, list(range(N_CORES)))
    out = np.stack([res.results[b]["y"].reshape(C, 32, 32) for b in range(N_CORES)])
    return out.astype(np.float32)
